# revision 2
# baseline (speedup 1.0000x reference)
"""Trainium2 Bass kernel v3 for nn_Block_65987877535901 (dense transformer).

Sharding: tensor-parallel over heads within each 4-core batch group.
Core c: batch g=c//4, rank r=c%4 owns heads 4r..4r+3.

v3 over v2:
  - software-pipelined attention kt loop (S matmul emitted ahead of the
    previous tile's AV so the in-order PE queue never head-of-line blocks
    on exp)
  - per-half pp tensors (no false WAR dependency between proj-half-1 DMA
    writes and the half-0 ReduceScatter)
  - the whole residual/LN2/MLP tail is split into two independent 256-token
    half-pipelines, so RS half 1 overlaps half 0's MLP
  - weight DMAs batched via 3D access patterns (1 DMA per fc weight group)
  - x DMAs issued first, on the scalar HWDGE queue
"""
import sys

sys.path.insert(0, "/opt/trn_rl_repo")

import numpy as np
import ml_dtypes

BF16 = ml_dtypes.bfloat16

P = 128
C = 1024
NCH = C // P      # 8
T = 2048
NT = 512          # own tokens per core
HT = 256          # tokens per half-pipeline
H = 16
HC = 4            # heads per core
HD = HC * 64      # 256 own head feats
D = 64
F = 4096
NFCH = F // P     # 32
NKT = T // P      # 16 key tiles
NQC = T // NT     # 4 query chunks / stat windows
NCORES = 8
EPS = 1e-5
GROUPS = [[0, 1, 2, 3], [4, 5, 6, 7]]

_COMPILED = None


def _build():
    import concourse.bacc as bacc
    import concourse.tile as tile
    import concourse.bass as bass
    from concourse import mybir

    dt = mybir.dt
    BF = dt.bfloat16
    F32 = dt.float32
    AF = mybir.ActivationFunctionType
    OP = mybir.AluOpType

    nc = bacc.Bacc("TRN2", target_bir_lowering=False, debug=False,
                   num_devices=NCORES)

    # ---- kernel I/O ----
    x = nc.declare_dram_parameter("x", [C, T], BF, isOutput=False)
    xres = nc.declare_dram_parameter("xres", [C, NT], F32, isOutput=False)
    wq = nc.declare_dram_parameter("wq", [C, HD], BF, isOutput=False)
    wk = nc.declare_dram_parameter("wk", [C, HD], BF, isOutput=False)
    wv = nc.declare_dram_parameter("wv", [C, HD], BF, isOutput=False)
    nwsq = nc.declare_dram_parameter("nwsq", [1, HD], BF, isOutput=False)
    nwsk = nc.declare_dram_parameter("nwsk", [1, HD], BF, isOutput=False)
    nwsv = nc.declare_dram_parameter("nwsv", [1, HD], BF, isOutput=False)
    bq = nc.declare_dram_parameter("bq", [HD], F32, isOutput=False)
    bk = nc.declare_dram_parameter("bk", [HD], F32, isOutput=False)
    bv = nc.declare_dram_parameter("bv", [HD], F32, isOutput=False)
    wp = nc.declare_dram_parameter("wp", [HD, C], BF, isOutput=False)
    bp = nc.declare_dram_parameter("bp", [C], F32, isOutput=False)
    wfc = nc.declare_dram_parameter("wfc", [C, F], BF, isOutput=False)
    bfc = nc.declare_dram_parameter("bfc", [F], F32, isOutput=False)
    wfc2 = nc.declare_dram_parameter("wfc2", [F, C], BF, isOutput=False)
    bfc2 = nc.declare_dram_parameter("bfc2", [C], F32, isOutput=False)
    maskd = nc.declare_dram_parameter("maskd", [P, 4 * 1024], BF,
                                      isOutput=False)
    out = nc.declare_dram_parameter("out", [C, NT], F32, isOutput=True)

    def chunk_col_ap(dram, nchunks):
        a = dram.ap()
        return bass.AP(tensor=a.tensor, offset=a.offset,
                       ap=[[1, P], [P, nchunks]])

    with tile.TileContext(nc) as tc:
        with (
            tc.tile_pool(name="const", bufs=1) as const,
            tc.tile_pool(name="persist", bufs=1) as persist,
            tc.tile_pool(name="dramp", bufs=1, space="DRAM") as dramp,
        ):
            # persistent activations
            kf = [persist.tile([P, T], BF, tag=f"kf{i}", name=f"kf{i}")
                  for i in range(2)]
            qf = [persist.tile([P, T], BF, tag=f"qf{i}", name=f"qf{i}")
                  for i in range(2)]
            vaug = [persist.tile([P, HC * (D + 1)], BF, tag=f"va{t}",
                                 name=f"va{t}") for t in range(NKT)]
            yf = [persist.tile([P, T], BF, tag=f"yf{i}", name=f"yf{i}")
                  for i in range(2)]
            abc = persist.tile([P, T], BF, tag="abc", name="abc")
            acol = persist.tile([P, NKT], F32, tag="acol", name="acol")
            mu_bfw = [persist.tile([1, NT], BF, tag=f"mubf{w}",
                                   name=f"mubf{w}") for w in range(NQC)]

            scrA = dramp.tile([NQC, NT], F32, tag="scrA", name="scrA")
            pp_in = [dramp.tile([NT, C], BF, tag=f"ppi{q}",
                                name=f"ppi{q}") for q in range(4)]
            pp_out = [dramp.tile([P, C], BF, tag=f"ppo{q}",
                                 name=f"ppo{q}") for q in range(4)]

            with (
                tc.tile_pool(name="xtp", bufs=1) as xtp,
                tc.tile_pool(name="lnw", bufs=1) as lnw,
            ):
                xt = [xtp.tile([P, T], BF, tag=f"xt{i}", name=f"xt{i}")
                      for i in range(NCH)]
                # x first, on the scalar HWDGE queue
                for i in range(NCH):
                    nc.scalar.dma_start(xt[i][:], x[i * P:(i + 1) * P, :])

                # ---------- constants (sync queue) ----------
                ones1 = const.tile([P, 1], BF, tag="ones1", name="ones1")
                nc.vector.memset(ones1[:], 1.0)
                onesr = const.tile([1, P], BF, tag="onesr", name="onesr")
                nc.vector.memset(onesr[:], 1.0)
                onesrf = const.tile([1, P], F32, tag="onesrf", name="onesrf")
                nc.vector.memset(onesrf[:], 1.0)
                epst = const.tile([P, 1], F32, tag="epst", name="epst")
                nc.vector.memset(epst[:], EPS)

                bq_col = const.tile([P, 2], F32, tag="bq_col", name="bq_col")
                nc.sync.dma_start(bq_col[:], chunk_col_ap(bq, 2))
                bk_col = const.tile([P, 2], F32, tag="bk_col", name="bk_col")
                nc.sync.dma_start(bk_col[:], chunk_col_ap(bk, 2))
                bp_col = const.tile([P, NCH], F32, tag="bp_col",
                                    name="bp_col")
                nc.sync.dma_start(bp_col[:], chunk_col_ap(bp, NCH))
                bfc_col = const.tile([P, NFCH], F32, tag="bfc_col",
                                     name="bfc_col")
                nc.sync.dma_start(bfc_col[:], chunk_col_ap(bfc, NFCH))
                bfc2_col = const.tile([P, NCH], F32, tag="bfc2_col",
                                      name="bfc2_col")
                nc.sync.dma_start(bfc2_col[:], chunk_col_ap(bfc2, NCH))
                bvbc = const.tile([P, HD], F32, tag="bvbc", name="bvbc")
                nc.sync.dma_start(
                    bvbc[:],
                    bass.AP(tensor=bv.ap().tensor, offset=bv.ap().offset,
                            ap=[[0, P], [1, HD]]))
                nwsq_sb = const.tile([1, HD], BF, tag="nwsq_sb",
                                     name="nwsq_sb")
                nc.sync.dma_start(nwsq_sb[:], nwsq[:, :])
                nwsk_sb = const.tile([1, HD], BF, tag="nwsk_sb",
                                     name="nwsk_sb")
                nc.sync.dma_start(nwsk_sb[:], nwsk[:, :])
                nwsv_sb = const.tile([1, HD], BF, tag="nwsv_sb",
                                     name="nwsv_sb")
                nc.sync.dma_start(nwsv_sb[:], nwsv[:, :])
                maskt = const.tile([P, 4 * 1024], BF, tag="maskt",
                                   name="maskt")
                nc.sync.dma_start(maskt[:], maskd[:, :])

                # ============ phase 1: LN1 stats ============
                s1g = lnw.tile([P, NT], F32, tag="s1g", name="s1g")
                s2g = lnw.tile([P, NT], F32, tag="s2g", name="s2g")
                nc.vector.memset(s1g[:], 1.0)
                nc.vector.memset(s2g[:], 1.0)
                with (
                    tc.tile_pool(name="sqp", bufs=2) as sqp,
                    tc.tile_pool(name="stps", bufs=1, space="PSUM") as stps,
                ):
                    st1 = stps.tile([1, T], F32, tag="st1", name="st1")
                    st2 = stps.tile([1, T], F32, tag="st2", name="st2")
                    for i in range(NCH):
                        sqt = sqp.tile([P, T], BF, tag="sqt", name="sqt")
                        nc.vector.tensor_mul(sqt[:], xt[i][:], xt[i][:])
                        for w in range(NQC):
                            nc.tensor.matmul(
                                st1[0:1, w * NT:(w + 1) * NT], ones1[:],
                                xt[i][:, w * NT:(w + 1) * NT],
                                start=(i == 0), stop=(i == NCH - 1))
                            nc.tensor.matmul(
                                st2[0:1, w * NT:(w + 1) * NT], ones1[:],
                                sqt[:, w * NT:(w + 1) * NT],
                                start=(i == 0), stop=(i == NCH - 1))
                    for w in range(NQC):
                        nc.vector.tensor_copy(s1g[32 * w:32 * w + 1, :],
                                              st1[0:1, w * NT:(w + 1) * NT])
                        nc.vector.tensor_copy(s2g[32 * w:32 * w + 1, :],
                                              st2[0:1, w * NT:(w + 1) * NT])
                mu_f = lnw.tile([P, NT], F32, tag="mu_f", name="mu_f")
                nc.vector.tensor_scalar_mul(mu_f[:], s1g[:], 1.0 / C)
                musq = lnw.tile([P, NT], F32, tag="musq", name="musq")
                nc.vector.tensor_mul(musq[:], mu_f[:], mu_f[:])
                var = lnw.tile([P, NT], F32, tag="var", name="var")
                nc.vector.scalar_tensor_tensor(
                    var[:], s2g[:], 1.0 / C, musq[:], OP.mult, OP.subtract)
                std = lnw.tile([P, NT], F32, tag="std", name="std")
                nc.scalar.activation(std[:], var[:], AF.Sqrt, bias=epst[:])
                a_full = lnw.tile([P, NT], F32, tag="a_full", name="a_full")
                nc.vector.reciprocal_approx_fast(a_full[:], std[:])
                a_bfw = [lnw.tile([1, NT], BF, tag=f"a_bfw{w}",
                                  name=f"a_bfw{w}") for w in range(NQC)]
                for w in range(NQC):
                    nc.vector.tensor_copy(mu_bfw[w][:],
                                          mu_f[32 * w:32 * w + 1, :])
                    nc.vector.tensor_copy(a_bfw[w][:],
                                          a_full[32 * w:32 * w + 1, :])
                with tc.tile_pool(name="bcps", bufs=1, space="PSUM") as bcps:
                    abcp = bcps.tile([P, T], F32, tag="abcp", name="abcp")
                    for w in range(NQC):
                        nc.tensor.matmul(abcp[:, w * NT:(w + 1) * NT],
                                         onesr[:], a_bfw[w][:],
                                         start=True, stop=True)
                    nc.vector.tensor_copy(abc[:], abcp[:])
                for w in range(NQC):
                    nc.sync.dma_start(scrA[w:w + 1, :],
                                      a_full[32 * w:32 * w + 1, :])
                sa = scrA[:]
                nc.sync.dma_start(
                    acol[:],
                    bass.AP(tensor=sa.tensor, offset=sa.offset,
                            ap=[[1, P], [P, NKT]]))

                # ============ phase 2+3: QKV + attention + proj ============
                with (
                    tc.tile_pool(name="wqkv", bufs=1) as wpool,
                    tc.tile_pool(name="projw", bufs=1) as pjw,
                    tc.tile_pool(name="psA", bufs=2, space="PSUM") as psA,
                    tc.tile_pool(name="psY", bufs=2, space="PSUM") as psY,
                    tc.tile_pool(name="epool", bufs=3) as epool,
                    tc.tile_pool(name="npool", bufs=2) as npool,
                    tc.tile_pool(name="ppsb", bufs=3) as ppsbp,
                ):
                    wkt = [wpool.tile([P, HD], BF, tag=f"wk{k}",
                                      name=f"wk{k}") for k in range(NCH)]
                    wqt = [wpool.tile([P, HD], BF, tag=f"wq{k}",
                                      name=f"wq{k}") for k in range(NCH)]
                    wvt = [wpool.tile([P, HD], BF, tag=f"wv{k}",
                                      name=f"wv{k}") for k in range(NCH)]
                    for k in range(NCH):
                        nc.sync.dma_start(wkt[k][:], wk[k * P:(k + 1) * P, :])
                        nc.sync.dma_start(wqt[k][:], wq[k * P:(k + 1) * P, :])
                        nc.sync.dma_start(wvt[k][:], wv[k * P:(k + 1) * P, :])
                    wpt = [pjw.tile([P, C], BF, tag=f"wp{f}", name=f"wp{f}")
                           for f in range(2)]
                    for f in range(2):
                        nc.sync.dma_start(wpt[f][:], wp[f * P:(f + 1) * P, :])

                    def ps_tile():
                        return psA.tile([P, 1024], F32, tag="ps", name="ps")

                    def kq_chunk(wt, nws_sb, bcol, dst, hp):
                        for th in range(2):
                            ps = ps_tile()
                            for k in range(NCH):
                                for j in range(2):
                                    nc.tensor.matmul(
                                        ps[:, j * NT:(j + 1) * NT],
                                        wt[k][:, hp * P:(hp + 1) * P],
                                        xt[k][:, (2 * th + j) * NT:
                                               (2 * th + j + 1) * NT],
                                        start=(k == 0), stop=False)
                            for j in range(2):
                                w = 2 * th + j
                                nc.tensor.matmul(
                                    ps[:, j * NT:(j + 1) * NT],
                                    nws_sb[0:1, hp * P:(hp + 1) * P],
                                    mu_bfw[w][:], start=False, stop=True)
                            sl = dst[hp][:, th * 1024:(th + 1) * 1024]
                            nc.vector.tensor_mul(
                                sl, ps[:], abc[:, th * 1024:(th + 1) * 1024])
                            nc.vector.tensor_scalar(
                                sl, sl, bcol[:, hp:hp + 1], None, OP.add)

                    def v_tile(kt):
                        ps = ps_tile()
                        psv = ps[:, 0:HD]
                        for k in range(NCH):
                            nc.tensor.matmul(psv,
                                             xt[k][:, kt * P:(kt + 1) * P],
                                             wvt[k][:], start=(k == 0),
                                             stop=False)
                        nc.tensor.matmul(
                            psv,
                            mu_bfw[kt // 4][0:1,
                                            (kt % 4) * P:(kt % 4 + 1) * P],
                            nwsv_sb[:], start=False, stop=True)
                        v3 = vaug[kt].rearrange("p (h x) -> p h x", h=HC)
                        nc.vector.scalar_tensor_tensor(
                            v3[:, :, 0:D],
                            psv.rearrange("p (h x) -> p h x", h=HC),
                            acol[:, kt:kt + 1],
                            bvbc[:].rearrange("p (h x) -> p h x", h=HC),
                            OP.mult, OP.add)
                        nc.vector.memset(v3[:, :, D:D + 1], 1.0)

                    def attn_block(hp, qc):
                        psy = psY.tile([D + 1, 1024], F32, tag="psy",
                                       name="psy")
                        nkt_q = 4 * qc + 4

                        def s_exp(kt):
                            pss = ps_tile()
                            for u in range(2):
                                nc.tensor.matmul(
                                    pss[:, u * NT:(u + 1) * NT],
                                    kf[hp][u * D:(u + 1) * D,
                                           kt * P:(kt + 1) * P],
                                    qf[hp][u * D:(u + 1) * D,
                                           qc * NT:(qc + 1) * NT],
                                    start=True, stop=True)
                            et = epool.tile([P, 1024], BF, tag="e", name="e")
                            nc.scalar.activation(et[:], pss[:], AF.Exp)
                            j = kt - 4 * qc
                            if j >= 0:
                                nc.vector.tensor_mul(
                                    et[:], et[:],
                                    maskt[:, j * 1024:(j + 1) * 1024])
                            return et

                        def av(kt, et):
                            for u in range(2):
                                lh = 2 * hp + u
                                nc.tensor.matmul(
                                    psy[:, u * NT:(u + 1) * NT],
                                    vaug[kt][:, lh * (D + 1):
                                             (lh + 1) * (D + 1)],
                                    et[:, u * NT:(u + 1) * NT],
                                    start=(kt == 0), stop=(kt == nkt_q - 1))

                        prev = s_exp(0)
                        for kt in range(1, nkt_q):
                            cur = s_exp(kt)
                            av(kt - 1, prev)
                            prev = cur
                        av(nkt_q - 1, prev)

                        rbs = []
                        for u in range(2):
                            dsb = npool.tile([1, NT], F32, tag=f"dsb{u}",
                                             name=f"dsb{u}")
                            nc.vector.tensor_copy(
                                dsb[:], psy[D:D + 1, u * NT:(u + 1) * NT])
                            rb = npool.tile([1, NT], F32, tag=f"rb{u}",
                                            name=f"rb{u}")
                            nc.vector.reciprocal_approx_fast(rb[:], dsb[:])
                            rbs.append(rb)
                        rt = ps_tile()
                        rbc_ps = rt[0:D, :]
                        for u in range(2):
                            nc.tensor.matmul(rbc_ps[:, u * NT:(u + 1) * NT],
                                             onesrf[0:1, 0:D], rbs[u][:],
                                             start=True, stop=True)
                        rbc_sb = npool.tile([D, 1024], F32, tag="rbcs",
                                            name="rbcs")
                        nc.vector.tensor_copy(rbc_sb[:], rbc_ps)
                        for u in range(2):
                            nc.vector.tensor_mul(
                                yf[hp][u * D:(u + 1) * D,
                                       qc * NT:(qc + 1) * NT],
                                psy[0:D, u * NT:(u + 1) * NT],
                                rbc_sb[:, u * NT:(u + 1) * NT])

                    def proj_tile(t):
                        ps = ps_tile()
                        for f in range(2):
                            for j in range(2):
                                nc.tensor.matmul(
                                    ps[:, j * NT:(j + 1) * NT],
                                    yf[f][:, t * P:(t + 1) * P],
                                    wpt[f][:, j * NT:(j + 1) * NT],
                                    start=(f == 0), stop=(f == 1))
                        pb = ppsbp.tile([P, C], BF, tag="ppsb", name="ppsb")
                        nc.vector.tensor_copy(pb[:], ps[:])
                        q = t // 4
                        nc.sync.dma_start(
                            pp_in[q][(t % 4) * P:(t % 4 + 1) * P, :], pb[:])

                    # ---- emission order: qc-major + quarter RS ----
                    kq_chunk(wkt, nwsk_sb, bk_col, kf, 0)
                    kq_chunk(wqt, nwsq_sb, bq_col, qf, 0)
                    for kt in range(4):
                        v_tile(kt)
                    attn_block(0, 0)
                    kq_chunk(wkt, nwsk_sb, bk_col, kf, 1)
                    kq_chunk(wqt, nwsq_sb, bq_col, qf, 1)
                    attn_block(1, 0)
                    for t in range(0, 4):
                        proj_tile(t)
                    nc.gpsimd.collective_compute(
                        "ReduceScatter", mybir.AluOpType.add,
                        replica_groups=GROUPS,
                        ins=[pp_in[0][:]], outs=[pp_out[0][:]])
                    for qc in range(1, NQC):
                        for kt in range(4 * qc, 4 * qc + 4):
                            v_tile(kt)
                        attn_block(0, qc)
                        attn_block(1, qc)
                        for t in range(4 * qc, 4 * qc + 4):
                            proj_tile(t)
                        nc.gpsimd.collective_compute(
                            "ReduceScatter", mybir.AluOpType.add,
                            replica_groups=GROUPS,
                            ins=[pp_in[qc][:]], outs=[pp_out[qc][:]])

            # ============ tail: two 256-token half-pipelines ============
            with (
                tc.tile_pool(name="tailp", bufs=1) as tailp,
                tc.tile_pool(name="xrp", bufs=1) as xrp,
            ):
                xrt = [xrp.tile([P, NT], F32, tag=f"xr{m}", name=f"xr{m}")
                       for m in range(NCH)]
                for m in range(NCH):
                    nc.scalar.dma_start(xrt[m][:], xres[m * P:(m + 1) * P, :])

                for h in range(2):
                    h1 = [tailp.tile([P, HT], F32, tag=f"h1_{h}_{m}",
                                     name=f"h1_{h}_{m}") for m in range(NCH)]
                    xn2 = [tailp.tile([P, HT], BF, tag=f"xn2_{h}_{m}",
                                      name=f"xn2_{h}_{m}")
                           for m in range(NCH)]
                    with (
                        tc.tile_pool(name=f"hpre{h}", bufs=3) as hprep,
                        tc.tile_pool(name=f"ln2w{h}", bufs=2) as lnp,
                        tc.tile_pool(name=f"ln2ps{h}", bufs=1,
                                     space="PSUM") as lnps,
                    ):
                        for m in range(NCH):
                            hpre = hprep.tile([P, HT], BF, tag="hpre",
                                              name="hpre")
                            for qq in range(2):
                                nc.scalar.dma_start_transpose(
                                    hpre[:, qq * P:(qq + 1) * P],
                                    pp_out[2 * h + qq][:,
                                                       m * P:(m + 1) * P])
                            nc.vector.scalar_tensor_tensor(
                                h1[m][:], hpre[:], bp_col[:, m:m + 1],
                                xrt[m][:, h * HT:(h + 1) * HT],
                                OP.add, OP.add)
                        # LN2 on this half's 256 tokens
                        stp = lnps.tile([1, 1024], F32, tag="stl",
                                        name="stl")
                        h1b = [lnp.tile([P, HT], BF, tag=f"h1b{m}",
                                        name=f"h1b{m}") for m in range(NCH)]
                        for i in range(NCH):
                            nc.vector.tensor_copy(h1b[i][:], h1[i][:])
                            sqi = lnp.tile([P, HT], BF, tag="sq2", name="sq2")
                            nc.vector.tensor_mul(sqi[:], h1b[i][:],
                                                 h1b[i][:])
                            nc.tensor.matmul(stp[0:1, 0:HT], ones1[:],
                                             h1b[i][:], start=(i == 0),
                                             stop=(i == NCH - 1))
                            nc.tensor.matmul(stp[0:1, NT:NT + HT],
                                             ones1[:],
                                             sqi[:], start=(i == 0),
                                             stop=(i == NCH - 1))
                        mu2 = lnp.tile([1, HT], F32, tag="mu2", name="mu2")
                        nc.vector.tensor_scalar_mul(mu2[:], stp[0:1, 0:HT],
                                                    1.0 / C)
                        musq2 = lnp.tile([1, HT], F32, tag="musq2",
                                         name="musq2")
                        nc.vector.tensor_mul(musq2[:], mu2[:], mu2[:])
                        var2 = lnp.tile([1, HT], F32, tag="var2",
                                        name="var2")
                        nc.vector.scalar_tensor_tensor(
                            var2[:], stp[0:1, NT:NT + HT], 1.0 / C, musq2[:],
                            OP.mult, OP.subtract)
                        std2 = lnp.tile([1, HT], F32, tag="std2",
                                        name="std2")
                        nc.scalar.activation(std2[:], var2[:], AF.Sqrt,
                                             bias=epst[0:1, :])
                        a2 = lnp.tile([1, HT], F32, tag="a2", name="a2")
                        nc.vector.reciprocal_approx_fast(a2[:], std2[:])
                        a2b = lnp.tile([1, HT], BF, tag="a2b", name="a2b")
                        nc.vector.tensor_copy(a2b[:], a2[:])
                        mu2b = lnp.tile([1, HT], BF, tag="mu2b",
                                        name="mu2b")
                        nc.vector.tensor_copy(mu2b[:], mu2[:])
                        a2bc = lnp.tile([P, HT], BF, tag="a2bc",
                                        name="a2bc")
                        mu2bc = lnp.tile([P, HT], BF, tag="mu2bc",
                                         name="mu2bc")
                        with tc.tile_pool(name=f"bc2ps{h}", bufs=1,
                                          space="PSUM") as bc2ps:
                            bps = bc2ps.tile([P, 1024], F32, tag="bps",
                                             name="bps")
                            nc.tensor.matmul(bps[:, 0:HT], onesr[:],
                                             a2b[:], start=True, stop=True)
                            nc.tensor.matmul(bps[:, NT:NT + HT], onesr[:],
                                             mu2b[:], start=True, stop=True)
                            nc.vector.tensor_copy(a2bc[:], bps[:, 0:HT])
                            nc.vector.tensor_copy(mu2bc[:],
                                                  bps[:, NT:NT + HT])
                        for i in range(NCH):
                            tt = lnp.tile([P, HT], BF, tag="tt2",
                                          name="tt2")
                            nc.vector.tensor_sub(tt[:], h1b[i][:], mu2bc[:])
                            nc.vector.tensor_mul(xn2[i][:], tt[:], a2bc[:])

                    # ---- MLP on this half: fc then fc2 ----
                    with tc.tile_pool(name=f"hmlpp{h}", bufs=1) as hmlpp:
                        hmlp = [hmlpp.tile([P, HT], BF, tag=f"hm{m}",
                                           name=f"hm{m}")
                                for m in range(NFCH)]
                        wfa = wfc.ap()
                        wf2a = wfc2.ap()
                        with (
                            tc.tile_pool(name=f"wfcp{h}", bufs=2) as wfcp,
                            tc.tile_pool(name=f"ps6{h}", bufs=2,
                                         space="PSUM") as ps6,
                        ):
                            for mg in range(NFCH // 4):
                                wtg = wfcp.tile([P, 8 * NT], BF, tag="wfcg",
                                                name="wfcg")
                                nc.sync.dma_start(
                                    wtg[:].rearrange("p (k c) -> p k c",
                                                     k=NCH),
                                    bass.AP(tensor=wfa.tensor,
                                            offset=wfa.offset + mg * 4 * P,
                                            ap=[[F, P], [P * F, NCH],
                                                [1, 4 * P]]))
                                psT = [ps6.tile([P, 1024], F32, tag="fc",
                                                name="fc")
                                       for _ in range(2)]

                                def fc_slot(mm):
                                    return psT[mm // 2][:, (mm % 2) * NT:
                                                        (mm % 2) * NT + HT]

                                for k in range(NCH):
                                    for mm in range(4):
                                        nc.tensor.matmul(
                                            fc_slot(mm),
                                            wtg[:, k * NT + mm * P:
                                                k * NT + (mm + 1) * P],
                                            xn2[k][:], start=(k == 0),
                                            stop=(k == NCH - 1))
                                for mm in range(4):
                                    m = 4 * mg + mm
                                    nc.scalar.activation(
                                        hmlp[m][:], fc_slot(mm), AF.Gelu,
                                        bias=bfc_col[:, m:m + 1])
                        with (
                            tc.tile_pool(name=f"wfc2p{h}", bufs=3) as wfc2p,
                            tc.tile_pool(name=f"ps7{h}", bufs=4,
                                         space="PSUM") as ps7,
                            tc.tile_pool(name=f"outp{h}", bufs=2) as outp,
                        ):
                            pso = [ps7.tile([P, 1024], F32, tag="fo",
                                            name="fo") for _ in range(4)]

                            def fo_slot(m):
                                return pso[m // 2][:, (m % 2) * NT:
                                                   (m % 2) * NT + HT]

                            for kg in range(NFCH // 4):
                                wt2g = wfc2p.tile([P, 4 * C], BF,
                                                  tag="wfc2g", name="wfc2g")
                                nc.sync.dma_start(
                                    wt2g[:].rearrange("p (k c) -> p k c",
                                                      k=4),
                                    bass.AP(tensor=wf2a.tensor,
                                            offset=wf2a.offset
                                            + kg * 4 * P * C,
                                            ap=[[C, P], [P * C, 4],
                                                [1, C]]))
                                for kk in range(4):
                                    k = 4 * kg + kk
                                    for m in range(NCH):
                                        nc.tensor.matmul(
                                            fo_slot(m),
                                            wt2g[:, kk * C + m * P:
                                                 kk * C + (m + 1) * P],
                                            hmlp[k][:], start=(k == 0),
                                            stop=(k == NFCH - 1))
                            for m in range(NCH):
                                ot = outp.tile([P, HT], F32, tag="ot",
                                               name="ot")
                                nc.vector.scalar_tensor_tensor(
                                    ot[:], fo_slot(m), bfc2_col[:, m:m + 1],
                                    h1[m][:], OP.add, OP.add)
                                nc.sync.dma_start(
                                    out[m * P:(m + 1) * P,
                                        h * HT:(h + 1) * HT], ot[:])

    nc.compile()
    return nc


def _tok_idx(r):
    """core-rank r owns tokens {512*q + 128*r .. +128} for q in 0..3"""
    return np.concatenate([512 * q + 128 * r + np.arange(128)
                           for q in range(4)])


def _host_prep(x, ln1_g, ln1_b, W_attn, b_attn, W_proj, b_proj,
               ln2_g, ln2_b, W_fc, b_fc, W_fc2, b_fc2):
    x = np.asarray(x, dtype=np.float32)
    W_attn = np.asarray(W_attn, dtype=np.float32)
    b_attn = np.asarray(b_attn, dtype=np.float32)
    g1 = np.asarray(ln1_g, np.float32)
    b1 = np.asarray(ln1_b, np.float32)
    g2 = np.asarray(ln2_g, np.float32)
    b2 = np.asarray(ln2_b, np.float32)
    W_fc = np.asarray(W_fc, np.float32)
    W_fc2 = np.asarray(W_fc2, np.float32)

    Wt = W_attn * g1[:, None]            # g1 folded into rows
    beff = b_attn + b1 @ W_attn          # b1 folded into bias

    wfc_t = (W_fc * g2[:, None]).astype(BF16)
    bfc_eff = (np.asarray(b_fc, np.float32) + b2 @ W_fc).astype(np.float32)

    # diagonal-tile masks: m[j][k, q] = (128*j + k <= q), dup for both heads
    kk = np.arange(P).reshape(P, 1)
    qq = np.arange(NT).reshape(1, NT)
    maskd = np.empty((P, 4 * 1024), dtype=BF16)
    for j in range(4):
        mj = (P * j + kk <= qq).astype(BF16)
        maskd[:, j * 1024:j * 1024 + NT] = mj
        maskd[:, j * 1024 + NT:(j + 1) * 1024] = mj

    shared = dict(
        wfc=wfc_t, bfc=bfc_eff,
        wfc2=W_fc2.astype(BF16),
        bfc2=np.asarray(b_fc2, np.float32),
        bp=np.asarray(b_proj, np.float32),
        maskd=maskd,
    )

    xT = [np.ascontiguousarray(x[b].T) for b in range(2)]   # [C, T] f32
    xTb = [t.astype(BF16) for t in xT]

    in_maps = []
    for c in range(NCORES):
        g, r = c // 4, c % 4
        hs = slice(HD * r, HD * r + HD)
        wq_ = (Wt[:, 0 * C:1 * C][:, hs] / 8.0).astype(BF16)
        wk_ = Wt[:, 1 * C:2 * C][:, hs].astype(BF16)
        wv_ = Wt[:, 2 * C:3 * C][:, hs].astype(BF16)
        tok = _tok_idx(r)
        m = dict(shared)
        m["x"] = xTb[g]
        m["xres"] = np.ascontiguousarray(xT[g][:, tok])
        m["wq"], m["wk"], m["wv"] = wq_, wk_, wv_
        m["nwsq"] = (-wq_.astype(np.float32).sum(0, keepdims=True)).astype(BF16)
        m["nwsk"] = (-wk_.astype(np.float32).sum(0, keepdims=True)).astype(BF16)
        m["nwsv"] = (-wv_.astype(np.float32).sum(0, keepdims=True)).astype(BF16)
        m["bq"] = (beff[0 * C:1 * C][hs] / 8.0).astype(np.float32)
        m["bk"] = beff[1 * C:2 * C][hs].astype(np.float32)
        m["bv"] = beff[2 * C:3 * C][hs].astype(np.float32)
        m["wp"] = np.ascontiguousarray(
            np.asarray(W_proj, np.float32)[hs, :]).astype(BF16)
        in_maps.append(m)
    return in_maps


def kernel(x, ln1_g, ln1_b, W_attn, b_attn, W_proj, b_proj,
           ln2_g, ln2_b, W_fc, b_fc, W_fc2, b_fc2):
    global _COMPILED
    from concourse.bass_utils import run_bass_kernel_spmd

    if _COMPILED is None:
        _COMPILED = _build()
    nc = _COMPILED
    in_maps = _host_prep(x, ln1_g, ln1_b, W_attn, b_attn, W_proj, b_proj,
                         ln2_g, ln2_b, W_fc, b_fc, W_fc2, b_fc2)
    res = run_bass_kernel_spmd(nc, in_maps, list(range(NCORES)))
    out = np.empty((2, T, C), dtype=np.float32)
    for c in range(NCORES):
        g, r = c // 4, c % 4
        out[g, _tok_idx(r), :] = res.results[c]["out"].T
    return out
